# revision 10
# baseline (speedup 1.0000x reference)
"""BiLSTM (B=32, S=512, I=H=1024) Trainium2 kernel over 8 NeuronCores.

Strategy (v2): tensor-parallel over the gate dimension — each core owns a
128-row H-slice and its four gate blocks for BOTH directions.  The two
directions are computed together each step:

  - PSUM gates tile (64, 512) fp32: partitions 0-31 = fwd batch,
    32-63 = bwd batch; free dim = [i|f|o|g] x 128 (this core's units).
    fwd accumulates on PE column-group 0 (tile_position (*,0)), bwd on
    column-group 1 (tile_position (*,32)) -> the two 9-matmul chains run
    CONCURRENTLY on disjoint 32-column strips of the PE array.
  - All matmul operands are bf16 (weights, h, xp, identities); PSUM/state
    stay fp32.  Native Tanh for g and c (sigmoid+tanh share an ACT table
    set, so no reload).
  - Per step ONE exchange of hT (128 units, 64 = fwd|bwd batch) bf16:
    either a gpsimd remote-DMA broadcast to the 7 peers (comm="rdma") or a
    single ncfw AllGather (comm="ag").
  - x-projection for all steps is computed on-device first (phase B) into
    DRAM xp (bf16), bias folded in.
"""

KERNEL_COMM = "ag"  # "rdma" | "ag"  (rdma: SWDGE remote-DMA faults under axon)
S_FIXED = 512
XPROJ_INTERLEAVE = True  # compute x-projection chunks inside the step loop
XPROJ_LEAD = 10          # chunks precomputed per end before the recurrence

LAST_EXEC_NS = None
LAST_RES = None

import numpy as np

import concourse.bass as bass
import concourse.bacc as bacc
import concourse.mybir as mybir
import concourse.tile as tile
from concourse import library_config
from concourse.tile_rust import add_dep_helper

# The axon client has no /dev/neuron*, so the driver's NC/routing maps are
# unavailable.  Relative-dest remote DMA descriptors don't bake these values
# into the NEFF, so a plausible identity map is fine for client-side
# validation and the simulator.
import concourse.libnrt as _libnrt

try:
    _libnrt.get_trn2_nc_mapping()
except Exception:
    _libnrt.get_trn2_nc_mapping = lambda: {(0, i): i for i in range(8)}
try:
    _libnrt.get_device_id_to_routing_id_mapping()
except Exception:
    _fake_rid_map = lambda: {i: i for i in range(16)}
    _libnrt.get_device_id_to_routing_id_mapping = _fake_rid_map
    import concourse.bass_interp as _bi
    import concourse.replica_groups as _rg

    _bi.get_device_id_to_routing_id_mapping = _fake_rid_map
    _rg.get_device_id_to_routing_id_mapping = _fake_rid_map

P = 128
B = 32
I_DIM = 1024
H_DIM = 1024
NCORES = 8
KCH = H_DIM // P          # 8 k-chunks of the hidden dim
GS = 4 * H_DIM // NCORES  # 512 gate rows per core per direction
F32 = mybir.dt.float32
BF16 = mybir.dt.bfloat16
SIG = mybir.ActivationFunctionType.Sigmoid
TANH = mybir.ActivationFunctionType.Tanh


def _bf16(a):
    import ml_dtypes

    return np.asarray(a, np.float32).astype(ml_dtypes.bfloat16)


def host_prep(x, W_ii, W_hi, b_i, W_ii_r, W_hi_r, b_i_r, S):
    """Build the 8 per-core input maps (everything bf16)."""
    x = np.asarray(x, np.float32)
    # xT[i, s*B+b] = x[b, s, i]
    xT = _bf16(np.ascontiguousarray(x.transpose(2, 1, 0).reshape(I_DIM, S * B)))

    def slices(W, bvec, core):
        # gate rows for this core, order [i|f|o|g] (no scaling: native tanh)
        rows_i = np.arange(core * P, core * P + P)
        rows = np.concatenate(
            [rows_i, H_DIM + rows_i, 3 * H_DIM + rows_i, 2 * H_DIM + rows_i]
        )
        Ws = np.asarray(W, np.float32)[rows, :]
        bs = np.asarray(bvec, np.float32)[rows]
        return _bf16(np.ascontiguousarray(Ws.T)), _bf16(bs.reshape(1, GS))

    id2 = _bf16(np.vstack([np.eye(B), np.eye(B)]))        # (64, 32)
    id64 = _bf16(np.eye(2 * B))                           # (64, 64)
    ones128 = _bf16(np.ones((1, P)))
    in_maps = []
    for c in range(NCORES):
        wiT_f, bias_f = slices(W_ii, b_i, c)
        whT_f, _ = slices(W_hi, b_i, c)
        wiT_b, bias_b = slices(W_ii_r, b_i_r, c)
        whT_b, _ = slices(W_hi_r, b_i_r, c)
        in_maps.append({
            "xT": xT,
            "wiT_f": wiT_f, "whT_f": whT_f, "bias_f": bias_f,
            "wiT_b": wiT_b, "whT_b": whT_b, "bias_b": bias_b,
            "id2": id2, "id64": id64, "ones128": ones128,
        })
    return in_maps


def host_assemble(results, S):
    """results[c]["out"]: (2, S, B, P) bf16 -> full (B, S, 2H) fp32."""
    out = np.empty((B, S, 2 * H_DIM), np.float32)
    for c in range(NCORES):
        o = np.asarray(results[c]["out"], np.float32)  # (2, S, B, P)
        out[:, :, c * P:(c + 1) * P] = o[0].transpose(1, 0, 2)
        out[:, :, H_DIM + c * P:H_DIM + (c + 1) * P] = o[1].transpose(1, 0, 2)
    return out


def build_kernel(S, comm="rdma", rel_wait=False):
    nc = bacc.Bacc(None)
    SB = S * B
    MCH = SB // P  # sb-chunks of 128 (= 4 timesteps each)

    xT_e = nc.declare_dram_parameter("xT", [I_DIM, SB], BF16, isOutput=False)
    w_e = {}
    for d in ("f", "b"):
        w_e["wiT_" + d] = nc.declare_dram_parameter("wiT_" + d, [I_DIM, GS], BF16, isOutput=False)
        w_e["whT_" + d] = nc.declare_dram_parameter("whT_" + d, [H_DIM, GS], BF16, isOutput=False)
        w_e["bias_" + d] = nc.declare_dram_parameter("bias_" + d, [1, GS], BF16, isOutput=False)
    id2_e = nc.declare_dram_parameter("id2", [2 * B, B], BF16, isOutput=False)
    id64_e = nc.declare_dram_parameter("id64", [2 * B, 2 * B], BF16, isOutput=False)
    ones_e = nc.declare_dram_parameter("ones128", [1, P], BF16, isOutput=False)
    out_e = nc.declare_dram_parameter("out", [2, S, B, P], BF16, isOutput=True)

    MCH_ALL = S // 4
    # one DRAM tensor per 4-timestep chunk -> exact producer/consumer deps
    xp_chunks = [
        nc.dram_tensor(f"xp_{m}", [2, 4, B, GS], BF16) for m in range(MCH_ALL)
    ]

    with tile.TileContext(nc) as tc:
        with (
            tc.tile_pool(name="const", bufs=1) as constp,
            tc.tile_pool(name="xsb", bufs=3) as xsbp,
            tc.tile_pool(name="xpt_st", bufs=3) as xpst,
            tc.tile_pool(name="psumB", bufs=2, space="PSUM") as psumB,
            tc.tile_pool(name="psumC", bufs=2, space="PSUM") as psumC,
            tc.tile_pool(name="psumT", bufs=2, space="PSUM") as psumT,
            tc.tile_pool(name="state", bufs=1) as statep,
            tc.tile_pool(name="step", bufs=3) as stepp,
            tc.tile_pool(name="dram", bufs=2, space="DRAM") as dramp,
        ):
            if comm == "rdma":
                nc.gpsimd.load_library(library_config.remote_dma)
            # ---- constants / weights in SBUF ----
            id2 = constp.tile([2 * B, B], BF16, tag="id2", name="id2")
            nc.sync.dma_start(id2[:], id2_e[:])
            id64 = constp.tile([2 * B, 2 * B], BF16, tag="id64", name="id64")
            nc.sync.dma_start(id64[:], id64_e[:])
            ones128 = constp.tile([1, P], BF16, tag="ones", name="ones")
            nc.sync.dma_start(ones128[:], ones_e[:])
            wiT = {}
            whT = {}
            biasT = {}
            for d in ("f", "b"):
                wiT[d] = constp.tile([P, KCH, GS], BF16, tag="wiT" + d, name="wiT" + d)
                nc.sync.dma_start(
                    wiT[d][:],
                    w_e["wiT_" + d][:].rearrange("(k p) g -> p k g", p=P),
                )
                whT[d] = constp.tile([P, KCH, GS], BF16, tag="whT" + d, name="whT" + d)
                nc.sync.dma_start(
                    whT[d][:],
                    w_e["whT_" + d][:].rearrange("(k p) g -> p k g", p=P),
                )
                biasT[d] = constp.tile([1, GS], BF16, tag="bias" + d, name="bias" + d)
                nc.sync.dma_start(biasT[d][:], w_e["bias_" + d][:])

            # ---- x_proj of one 4-timestep chunk into DRAM (bias folded) ----
            def xproj_chunk(m):
                xsb = xsbp.tile([P, KCH, P], BF16, tag="xsb", name="xsb")
                nc.sync.dma_start(
                    xsb[:],
                    xT_e[:, m * P:(m + 1) * P].rearrange("(k p) c -> p k c", p=P),
                )
                for d in ("f", "b"):
                    ps = psumB.tile([P, GS], F32, tag="psB", name="psB")
                    nc.tensor.matmul(ps[:], ones128[:], biasT[d][:],
                                     start=True, stop=False)
                    for k in range(KCH):
                        nc.tensor.matmul(ps[:], xsb[:, k, :], wiT[d][:, k, :],
                                         start=False, stop=(k == KCH - 1))
                    xpt = xpst.tile([P, GS], BF16, tag="xpt", name="xpt")
                    nc.vector.tensor_copy(xpt[:], ps[:])
                    di = 0 if d == "f" else 1
                    nc.sync.dma_start(
                        xp_chunks[m][di].rearrange("s b g -> (s b) g"),
                        xpt[:],
                    )

            # chunk production order: LEAD chunks from each end up front, the
            # rest interleaved into the recurrence (front/back alternating)
            lead = min(XPROJ_LEAD, (MCH_ALL + 1) // 2) if XPROJ_INTERLEAVE else 0
            if not XPROJ_INTERLEAVE:
                for m in range(MCH_ALL):
                    xproj_chunk(m)
            else:
                for i in range(lead):
                    xproj_chunk(i)
                    if MCH_ALL - 1 - i > lead - 1:
                        xproj_chunk(MCH_ALL - 1 - i)
                pending = list(range(lead, MCH_ALL - lead))
                # alternate front/back consumption order
                prod_order = []
                lo, hi = 0, len(pending) - 1
                while lo <= hi:
                    prod_order.append(pending[lo]); lo += 1
                    if lo <= hi:
                        prod_order.append(pending[hi]); hi -= 1
                prod_iter = iter(prod_order)

            # ---- phase C: recurrence, both dirs together ----
            c_state = statep.tile([2 * B, P], F32, tag="c", name="c")
            nc.vector.memset(c_state[:], 0.0)

            if comm == "rdma":
                recv_sem = nc.alloc_semaphore("recv_sem")
                rel_sem = nc.alloc_semaphore("rel_sem")
                pid_rv = nc.gpsimd.partition_id()
                pid_dve = nc.vector.partition_id()
                RDESTS = [None] + [(0, j) for j in range(1, NCORES)]
                last_trigger = [None, None]
                prev_transpose = [None]
            hT_self_bufs = [
                statep.tile([P, 2 * B], BF16, tag=f"hTs{p}", name=f"hTs{p}")
                for p in (0, 1)
            ]
            hT_recv_bufs = [
                statep.tile([P, KCH, 2 * B], BF16, tag=f"hTr{p}", name=f"hTr{p}")
                for p in (0, 1)
            ]
            hT_prev = [None]

            def step(t):
                # xp for fwd step t and bwd step t (source position S-1-t)
                sb = S - 1 - t
                xpt = stepp.tile([2 * B, GS], BF16, tag="xp_t", name="xp_t")
                nc.sync.dma_start(xpt[0:B, :], xp_chunks[t // 4][0, t % 4])
                nc.sync.dma_start(xpt[B:2 * B, :], xp_chunks[sb // 4][1, sb % 4])
                ps = psumC.tile([2 * B, GS], F32, tag="psC", name="psC")
                # xp injection via identity matmuls (row strips 0 / 1)
                nc.tensor.matmul(ps[0:B, :], id2[0:B, :], xpt[0:B, :],
                                 start=True, stop=(t == 0),
                                 tile_position=(0, 0))
                nc.tensor.matmul(ps[B:2 * B, :], id2[B:2 * B, :], xpt[B:2 * B, :],
                                 start=True, stop=(t == 0),
                                 tile_position=(B, B))
                if t > 0:
                    hT = hT_recv_bufs[(t - 1) % 2]
                    if comm == "rdma":
                        wait_i = nc.tensor.wait_ge(recv_sem, 14 * t)
                        if prev_transpose[0] is not None:
                            add_dep_helper(
                                wait_i.ins, prev_transpose[0].ins, sync=False,
                                reason="rdma: recv-wait after own transpose")
                    for k in range(KCH):
                        mm = nc.tensor.matmul(
                            ps[0:B, :], hT[:, k, 0:B], whT["f"][:, k, :],
                            start=False, stop=(k == KCH - 1),
                            tile_position=(0, 0))
                        if comm == "rdma" and k == 0:
                            add_dep_helper(mm.ins, wait_i.ins, sync=False,
                                           reason="rdma: matmul after recv-wait")
                        nc.tensor.matmul(
                            ps[B:2 * B, :], hT[:, k, B:2 * B], whT["b"][:, k, :],
                            start=False, stop=(k == KCH - 1),
                            tile_position=(0, B))
                # activations: [i|f|o] sigmoid, [g] tanh
                acts = stepp.tile([2 * B, GS], F32, tag="acts", name="acts")
                nc.scalar.activation(acts[:, 0:3 * P], ps[:, 0:3 * P], SIG)
                nc.scalar.activation(acts[:, 3 * P:4 * P], ps[:, 3 * P:4 * P], TANH)
                i_ap = acts[:, 0 * P:1 * P]
                f_ap = acts[:, 1 * P:2 * P]
                o_ap = acts[:, 2 * P:3 * P]
                g_ap = acts[:, 3 * P:4 * P]
                u = stepp.tile([2 * B, P], F32, tag="u", name="u")
                nc.vector.tensor_mul(u[:], i_ap, g_ap)
                v = stepp.tile([2 * B, P], F32, tag="v", name="v")
                nc.vector.tensor_mul(v[:], f_ap, c_state[:])
                nc.vector.tensor_add(c_state[:], u[:], v[:])
                tc_t = stepp.tile([2 * B, P], F32, tag="tc", name="tc")
                nc.scalar.activation(tc_t[:], c_state[:], TANH)
                h = stepp.tile([2 * B, P], BF16, tag="h", name="h")
                nc.vector.tensor_mul(h[:], o_ap, tc_t[:])
                nc.sync.dma_start(out_e[0, t], h[0:B, :])
                nc.sync.dma_start(out_e[1, S - 1 - t], h[B:2 * B, :])
                if t == S - 1:
                    return
                # transpose h -> (P, 2B) and publish to peers
                tp = psumT.tile([P, 2 * B], BF16, tag="tp", name="tp")
                tp_inst = nc.tensor.transpose(tp[:], h[:], id64[:])
                hT_self = hT_self_bufs[t % 2]
                if comm == "rdma":
                    prev_transpose[0] = tp_inst
                    if rel_wait and t >= 2:
                        wr = nc.vector.wait_ge(rel_sem, 16 * (t - 1))
                    cp = nc.vector.tensor_copy(hT_self[:], tp[:])
                    if rel_wait and t >= 2:
                        add_dep_helper(cp.ins, wr.ins, sync=False,
                                       reason="rdma: copy after release wait")
                    if last_trigger[t % 2] is not None:
                        # WAR vs the SDMA read two steps ago; real safety comes
                        # from recv-sem transitivity, this orders the Tile
                        # schedule / race model.
                        add_dep_helper(cp.ins, last_trigger[t % 2].ins,
                                       sync=True,
                                       reason="rdma: reuse hT_self after trigger")
                    nc.vector.tensor_copy(
                        hT_recv_bufs[t % 2][:, bass.ds(pid_dve, 1), :],
                        hT_self[:])
                    nc.gpsimd.remote_dma_broadcast(
                        out_ap=hT_recv_bufs[t % 2][:, bass.ds(pid_rv, 1), :],
                        in_ap=hT_self[:],
                        remote_sem=recv_sem,
                        local_sem=rel_sem,
                        rdests=RDESTS,
                    )
                    trig = nc.gpsimd.trigger_dma(count=None)
                    last_trigger[t % 2] = trig
                    # Scheduler-sim-only stand-ins for the remote/SWDGE
                    # increments; stripped from the final BIR.
                    nc.gpsimd.sem_inc(recv_sem, 14)
                    nc.gpsimd.sem_inc(rel_sem, 16)
                else:  # ag
                    nc.vector.tensor_copy(hT_self[:], tp[:])
                    cc_in = dramp.tile([P, 2 * B], BF16, tag="ccin", name="ccin")
                    nc.sync.dma_start(cc_in[:], hT_self[:])
                    cc_out = dramp.tile([NCORES * P, 2 * B], BF16, tag="ccout",
                                        name="ccout")
                    nc.gpsimd.collective_compute(
                        "AllGather",
                        mybir.AluOpType.bypass,
                        ins=[cc_in[:].opt()],
                        outs=[cc_out[:].opt()],
                        replica_groups=[list(range(NCORES))],
                    )
                    nc.sync.dma_start(
                        hT_recv_bufs[t % 2][:],
                        cc_out[:].rearrange("(k p) b -> p k b", p=P),
                    )

            for t in range(S):
                step(t)
                if XPROJ_INTERLEAVE and t % 2 == 0:
                    m = next(prod_iter, None)
                    if m is not None:
                        xproj_chunk(m)

    if comm == "rdma":
        _strip_fake_incs(nc, ("recv_sem", "rel_sem"))
    return nc


def _strip_fake_incs(nc, sem_names):
    """Remove every on_update entry for the given sems (scheduler-sim-only
    stand-ins for remote increments) and drop update-only EventSemaphore
    carriers that become empty."""
    names = set(sem_names)
    for fn in nc.m.functions:
        for bb in fn.blocks:
            new = []
            changed = False
            for ins in bb.instructions:
                si = ins.sync_info
                if si is not None and any(
                    u.ant_name in names for u in si.on_update
                ):
                    kept = [u for u in si.on_update if u.ant_name not in names]
                    ins.sync_info = mybir.SyncInfo(
                        on_wait=list(si.on_wait), on_update=kept
                    )
                    changed = True
                    if (
                        isinstance(ins, mybir.InstEventSemaphore)
                        and not kept
                        and not si.on_wait
                    ):
                        continue  # drop the empty carrier
                new.append(ins)
            if changed:
                try:
                    bb.instructions = new
                except Exception:
                    bb.instructions.clear()
                    bb.instructions.extend(new)


def fix_drain_waits(nc):
    """This walrus build allows only 1 sync-wait per instruction (2 on
    EventSemaphore).  Move excess waits onto EventSemaphore insts placed
    immediately before the instruction on the same engine."""
    ctr = 0
    for fn in nc.m.functions:
        for bb in fn.blocks:
            insts = list(bb.instructions)
            new = []
            changed = False
            for ins in insts:
                si = ins.sync_info
                if (
                    not isinstance(ins, mybir.InstEventSemaphore)
                    and si is not None
                    and len(si.on_wait) > 1
                ):
                    waits = list(si.on_wait)
                    keep, extra = waits[:1], waits[1:]
                    for i in range(0, len(extra), 2):
                        w = mybir.InstEventSemaphore(
                            name=f"I-dwfix-{ctr}",
                            engine=ins.engine,
                            ins=[],
                            outs=[],
                            sync_info=mybir.SyncInfo(
                                on_wait=extra[i : i + 2], on_update=[]
                            ),
                        )
                        ctr += 1
                        new.append(w)
                    ins.sync_info = mybir.SyncInfo(
                        on_wait=keep, on_update=list(si.on_update)
                    )
                    changed = True
                new.append(ins)
            if changed:
                try:
                    bb.instructions = new
                except Exception:
                    bb.instructions.clear()
                    bb.instructions.extend(new)


def kernel(x, W_ii, W_hi, b_i, W_ii_reverse, W_hi_reverse, b_i_reverse):
    """Full inputs in, full (B, S, 2H) fp32 output out."""
    import os

    global LAST_EXEC_NS, LAST_RES
    import concourse.bass_utils as bu

    bu.upload_artifacts = lambda tmpdir: "local://" + tmpdir
    from concourse.bass_utils import run_bass_kernel_spmd

    S = S_FIXED
    comm = os.environ.get("TRNLSTM_COMM", KERNEL_COMM)
    trace = os.environ.get("TRNLSTM_TRACE", "0") == "1"

    nc = build_kernel(S, comm=comm)
    nc.compile()
    fix_drain_waits(nc)
    in_maps = host_prep(x, W_ii, W_hi, b_i,
                        W_ii_reverse, W_hi_reverse, b_i_reverse, S)
    res = run_bass_kernel_spmd(nc, in_maps, list(range(NCORES)), trace=trace)
    LAST_EXEC_NS = res.exec_time_ns
    LAST_RES = res
    return host_assemble(res.results, S)


# revision 23
# speedup vs baseline: 1.0242x; 1.0242x over previous
"""BiLSTM (B=32, S=512, I=H=1024) Trainium2 kernel over 8 NeuronCores.

Strategy (v2): tensor-parallel over the gate dimension — each core owns a
128-row H-slice and its four gate blocks for BOTH directions.  The two
directions are computed together each step:

  - PSUM gates tile (64, 512) fp32: partitions 0-31 = fwd batch,
    32-63 = bwd batch; free dim = [i|f|o|g] x 128 (this core's units).
    fwd accumulates on PE column-group 0 (tile_position (*,0)), bwd on
    column-group 1 (tile_position (*,32)) -> the two 9-matmul chains run
    CONCURRENTLY on disjoint 32-column strips of the PE array.
  - All matmul operands are bf16 (weights, h, xp, identities); PSUM/state
    stay fp32.  Native Tanh for g and c (sigmoid+tanh share an ACT table
    set, so no reload).
  - Per step ONE exchange of hT (128 units, 64 = fwd|bwd batch) bf16:
    either a gpsimd remote-DMA broadcast to the 7 peers (comm="rdma") or a
    single ncfw AllGather (comm="ag").
  - x-projection for all steps is computed on-device first (phase B) into
    DRAM xp (bf16), bias folded in.
"""

KERNEL_COMM = "ag"  # "rdma" | "ag"  (rdma: SWDGE remote-DMA faults under axon)
S_FIXED = 512
XPROJ_INTERLEAVE = True  # compute x-projection chunks inside the step loop
XPROJ_LEAD = 10          # chunks precomputed per end before the recurrence
STREAMS = 1              # independent batch streams; >1 does NOT help: each
                         # stream still needs S sequential iterations, so total
                         # time = S * loop_latency regardless (measured)

LAST_EXEC_NS = None
LAST_RES = None

import numpy as np

import concourse.bass as bass
import concourse.bacc as bacc
import concourse.mybir as mybir
import concourse.tile as tile
from concourse import library_config
from concourse.tile_rust import add_dep_helper

# The axon client has no /dev/neuron*, so the driver's NC/routing maps are
# unavailable.  Relative-dest remote DMA descriptors don't bake these values
# into the NEFF, so a plausible identity map is fine for client-side
# validation and the simulator.
import concourse.libnrt as _libnrt

try:
    _libnrt.get_trn2_nc_mapping()
except Exception:
    _libnrt.get_trn2_nc_mapping = lambda: {(0, i): i for i in range(8)}
try:
    _libnrt.get_device_id_to_routing_id_mapping()
except Exception:
    _fake_rid_map = lambda: {i: i for i in range(16)}
    _libnrt.get_device_id_to_routing_id_mapping = _fake_rid_map
    import concourse.bass_interp as _bi
    import concourse.replica_groups as _rg

    _bi.get_device_id_to_routing_id_mapping = _fake_rid_map
    _rg.get_device_id_to_routing_id_mapping = _fake_rid_map

P = 128
B = 32
I_DIM = 1024
H_DIM = 1024
NCORES = 8
KCH = H_DIM // P          # 8 k-chunks of the hidden dim
GS = 4 * H_DIM // NCORES  # 512 gate rows per core per direction
F32 = mybir.dt.float32
BF16 = mybir.dt.bfloat16
SIG = mybir.ActivationFunctionType.Sigmoid
TANH = mybir.ActivationFunctionType.Tanh


def _bf16(a):
    import ml_dtypes

    return np.asarray(a, np.float32).astype(ml_dtypes.bfloat16)


def host_prep(x, W_ii, W_hi, b_i, W_ii_r, W_hi_r, b_i_r, S):
    """Build the 8 per-core input maps (everything bf16)."""
    x = np.asarray(x, np.float32)
    # xT[i, s*B+b] = x[b, s, i]
    xT = _bf16(np.ascontiguousarray(x.transpose(2, 1, 0).reshape(I_DIM, S * B)))

    def slices(W, bvec, core):
        # gate rows for this core, order [i|f|o|g] (no scaling: native tanh)
        rows_i = np.arange(core * P, core * P + P)
        rows = np.concatenate(
            [rows_i, H_DIM + rows_i, 3 * H_DIM + rows_i, 2 * H_DIM + rows_i]
        )
        Ws = np.asarray(W, np.float32)[rows, :]
        bs = np.asarray(bvec, np.float32)[rows]
        return _bf16(np.ascontiguousarray(Ws.T)), _bf16(bs.reshape(1, GS))

    # injection identity: strip s (rows 0.. / 32..) holds [I_SM | 0] so the
    # M=32-wide inject also zero-fills junk rows when STREAMS=2
    SM = B // STREAMS
    id2 = np.zeros((2 * B, B), np.float32)
    id2[0:SM, 0:SM] = np.eye(SM)
    id2[B:B + SM, 0:SM] = np.eye(SM)
    if STREAMS == 1:
        pass  # SM == B: strips fully identity, same as before
    id2 = _bf16(id2)
    id64 = _bf16(np.eye(2 * B))                           # (64, 64)
    ones128 = _bf16(np.ones((1, P)))
    in_maps = []
    for c in range(NCORES):
        wiT_f, bias_f = slices(W_ii, b_i, c)
        whT_f, _ = slices(W_hi, b_i, c)
        wiT_b, bias_b = slices(W_ii_r, b_i_r, c)
        whT_b, _ = slices(W_hi_r, b_i_r, c)
        in_maps.append({
            "xT": xT,
            "wiT_f": wiT_f, "whT_f": whT_f, "bias_f": bias_f,
            "wiT_b": wiT_b, "whT_b": whT_b, "bias_b": bias_b,
            "id2": id2, "id64": id64, "ones128": ones128,
        })
    return in_maps


def host_assemble(results, S):
    """results[c]["out"]: (2, S, B, P) bf16 -> full (B, S, 2H) fp32."""
    out = np.empty((B, S, 2 * H_DIM), np.float32)
    for c in range(NCORES):
        o = np.asarray(results[c]["out"], np.float32)  # (2, S, B, P)
        out[:, :, c * P:(c + 1) * P] = o[0].transpose(1, 0, 2)
        out[:, :, H_DIM + c * P:H_DIM + (c + 1) * P] = o[1].transpose(1, 0, 2)
    return out


def build_kernel(S, comm="rdma", rel_wait=False):
    nc = bacc.Bacc(None)
    SB = S * B
    MCH = SB // P  # sb-chunks of 128 (= 4 timesteps each)

    xT_e = nc.declare_dram_parameter("xT", [I_DIM, SB], BF16, isOutput=False)
    w_e = {}
    for d in ("f", "b"):
        w_e["wiT_" + d] = nc.declare_dram_parameter("wiT_" + d, [I_DIM, GS], BF16, isOutput=False)
        w_e["whT_" + d] = nc.declare_dram_parameter("whT_" + d, [H_DIM, GS], BF16, isOutput=False)
        w_e["bias_" + d] = nc.declare_dram_parameter("bias_" + d, [1, GS], BF16, isOutput=False)
    id2_e = nc.declare_dram_parameter("id2", [2 * B, B], BF16, isOutput=False)
    id64_e = nc.declare_dram_parameter("id64", [2 * B, 2 * B], BF16, isOutput=False)
    ones_e = nc.declare_dram_parameter("ones128", [1, P], BF16, isOutput=False)
    out_e = nc.declare_dram_parameter("out", [2, S, B, P], BF16, isOutput=True)

    MCH_ALL = S // 4
    # one DRAM tensor per 4-timestep chunk -> exact producer/consumer deps
    xp_chunks = [
        nc.dram_tensor(f"xp_{m}", [2, 4, B, GS], BF16) for m in range(MCH_ALL)
    ]

    with tile.TileContext(nc) as tc:
        with (
            tc.tile_pool(name="const", bufs=1) as constp,
            tc.tile_pool(name="xsb", bufs=3) as xsbp,
            tc.tile_pool(name="xpt_st", bufs=3) as xpst,
            tc.tile_pool(name="psumB", bufs=2, space="PSUM") as psumB,
            tc.tile_pool(name="psumC", bufs=2, space="PSUM") as psumC,
            tc.tile_pool(name="psumT", bufs=1, space="PSUM") as psumT,
            tc.tile_pool(name="state", bufs=1) as statep,
            tc.tile_pool(name="step", bufs=3) as stepp,
            tc.tile_pool(name="dram", bufs=2, space="DRAM") as dramp,
        ):
            if comm == "rdma":
                nc.gpsimd.load_library(library_config.remote_dma)
            # ---- constants / weights in SBUF ----
            idU = constp.tile([2 * B, B], BF16, tag="id2", name="id2")
            nc.sync.dma_start(idU[:], id2_e[:])
            id64 = constp.tile([2 * B, 2 * B], BF16, tag="id64", name="id64")
            nc.sync.dma_start(id64[:], id64_e[:])
            ones128 = constp.tile([1, P], BF16, tag="ones", name="ones")
            nc.sync.dma_start(ones128[:], ones_e[:])
            wiT = {}
            whT = {}
            biasT = {}
            for d in ("f", "b"):
                wiT[d] = constp.tile([P, KCH, GS], BF16, tag="wiT" + d, name="wiT" + d)
                nc.sync.dma_start(
                    wiT[d][:],
                    w_e["wiT_" + d][:].rearrange("(k p) g -> p k g", p=P),
                )
                whT[d] = constp.tile([P, KCH, GS], BF16, tag="whT" + d, name="whT" + d)
                nc.sync.dma_start(
                    whT[d][:],
                    w_e["whT_" + d][:].rearrange("(k p) g -> p k g", p=P),
                )
                biasT[d] = constp.tile([1, GS], BF16, tag="bias" + d, name="bias" + d)
                nc.sync.dma_start(biasT[d][:], w_e["bias_" + d][:])

            # ---- x_proj of one 4-timestep chunk into DRAM (bias folded) ----
            _xsb_cur = {}

            def xproj_half(m, d):
                if d == "f":
                    xsb = xsbp.tile([P, KCH, P], BF16, tag="xsb", name="xsb")
                    nc.sync.dma_start(
                        xsb[:],
                        xT_e[:, m * P:(m + 1) * P].rearrange(
                            "(k p) c -> p k c", p=P),
                    )
                    _xsb_cur[m] = xsb
                else:
                    xsb = _xsb_cur.pop(m)
                ps = psumB.tile([P, GS], F32, tag="psB", name="psB")
                nc.tensor.matmul(ps[:], ones128[:], biasT[d][:],
                                 start=True, stop=False)
                for k in range(KCH):
                    nc.tensor.matmul(ps[:], xsb[:, k, :], wiT[d][:, k, :],
                                     start=False, stop=(k == KCH - 1))
                xpt = xpst.tile([P, GS], BF16, tag="xpt", name="xpt")
                nc.vector.tensor_copy(xpt[:], ps[:])
                di = 0 if d == "f" else 1
                nc.sync.dma_start(
                    xp_chunks[m][di].rearrange("s b g -> (s b) g"),
                    xpt[:],
                )

            def xproj_chunk(m):
                xproj_half(m, "f")
                xproj_half(m, "b")

            # chunk production order: LEAD chunks from each end up front, the
            # rest interleaved into the recurrence (front/back alternating)
            lead = min(XPROJ_LEAD, (MCH_ALL + 1) // 2) if XPROJ_INTERLEAVE else 0
            if not XPROJ_INTERLEAVE:
                for m in range(MCH_ALL):
                    xproj_chunk(m)
            else:
                for i in range(lead):
                    xproj_chunk(i)
                    if MCH_ALL - 1 - i > lead - 1:
                        xproj_chunk(MCH_ALL - 1 - i)
                pending = list(range(lead, MCH_ALL - lead))
                # alternate front/back consumption order
                prod_order = []
                lo, hi = 0, len(pending) - 1
                while lo <= hi:
                    prod_order.append(pending[lo]); lo += 1
                    if lo <= hi:
                        prod_order.append(pending[hi]); hi -= 1
                prod_iter = iter(
                    [(m, d) for m in prod_order for d in ("f", "b")])

            # ---- phase C: recurrence, both dirs together, NQ streams ----
            NQ = STREAMS
            SM = B // NQ  # batch rows per stream per direction
            c_state = {}
            hT_self_bufs = {}
            hT_recv_bufs = {}
            for q in range(NQ):
                c_state[q] = statep.tile([2 * B, P], F32, tag=f"c{q}",
                                         name=f"c{q}")
                nc.vector.memset(c_state[q][:], 0.0)
                hT_self_bufs[q] = [
                    statep.tile([P, 2 * B], BF16, tag=f"hTs{q}{p}",
                                name=f"hTs{q}{p}") for p in (0, 1)
                ]
                hT_recv_bufs[q] = [
                    statep.tile([P, KCH, 2 * B], BF16, tag=f"hTr{q}{p}",
                                name=f"hTr{q}{p}") for p in (0, 1)
                ]

            def step(q, t):
                # xp for fwd step t and bwd step t (source position S-1-t),
                # this stream's batch slice; junk lanes are zero-filled by the
                # M=32-wide injection matmuls so downstream math stays finite.
                sb = S - 1 - t
                bs = slice(SM * q, SM * (q + 1))
                xpt = stepp.tile([2 * B, GS], BF16, tag=f"xp_t{q}",
                                 name=f"xp_t{q}")
                nc.sync.dma_start(xpt[0:SM, :], xp_chunks[t // 4][0, t % 4, bs])
                nc.sync.dma_start(xpt[B:B + SM, :],
                                  xp_chunks[sb // 4][1, sb % 4, bs])
                ps = psumC.tile([2 * B, GS], F32, tag=f"psC{q}", name=f"psC{q}")
                # xp injection via identity matmuls (row strips 0 / 1);
                # idU is [I_SM | 0] per strip -> writes zeros to junk rows
                nc.tensor.matmul(ps[0:B, :], idU[0:SM, :], xpt[0:SM, :],
                                 start=True, stop=(t == 0),
                                 tile_position=(0, 0))
                nc.tensor.matmul(ps[B:2 * B, :], idU[B:B + SM, :],
                                 xpt[B:B + SM, :],
                                 start=True, stop=(t == 0),
                                 tile_position=(B, B))
                if t > 0:
                    hT = hT_recv_bufs[q][(t - 1) % 2]
                    for k in range(KCH):
                        nc.tensor.matmul(
                            ps[0:SM, :], hT[:, k, 0:SM], whT["f"][:, k, :],
                            start=False, stop=(k == KCH - 1),
                            tile_position=(0, 0))
                        nc.tensor.matmul(
                            ps[B:B + SM, :], hT[:, k, B:B + SM],
                            whT["b"][:, k, :],
                            start=False, stop=(k == KCH - 1),
                            tile_position=(0, B))
                # activations: g(tanh) and i(sig) first so the DVE chain can
                # start while [f|o] sigmoids run
                acts = stepp.tile([2 * B, GS], F32, tag=f"acts{q}",
                                  name=f"acts{q}")
                nc.scalar.activation(acts[:, 3 * P:4 * P], ps[:, 3 * P:4 * P],
                                     TANH)
                nc.scalar.activation(acts[:, 0:P], ps[:, 0:P], SIG)
                nc.scalar.activation(acts[:, P:3 * P], ps[:, P:3 * P], SIG)
                i_ap = acts[:, 0 * P:1 * P]
                f_ap = acts[:, 1 * P:2 * P]
                o_ap = acts[:, 2 * P:3 * P]
                g_ap = acts[:, 3 * P:4 * P]
                u = stepp.tile([2 * B, P], F32, tag=f"u{q}", name=f"u{q}")
                nc.vector.tensor_mul(u[:], i_ap, g_ap)
                v = stepp.tile([2 * B, P], F32, tag=f"v{q}", name=f"v{q}")
                nc.vector.tensor_mul(v[:], f_ap, c_state[q][:])
                nc.vector.tensor_add(c_state[q][:], u[:], v[:])
                tc_t = stepp.tile([2 * B, P], F32, tag=f"tc{q}", name=f"tc{q}")
                nc.scalar.activation(tc_t[:], c_state[q][:], TANH)
                h = stepp.tile([2 * B, P], BF16, tag=f"h{q}", name=f"h{q}")
                nc.vector.tensor_mul(h[:], o_ap, tc_t[:])
                nc.sync.dma_start(out_e[0, t, bs], h[0:SM, :])
                nc.sync.dma_start(out_e[1, S - 1 - t, bs], h[B:B + SM, :])
                if t == S - 1:
                    return
                # transpose h -> (P, 2B) and publish to peers
                tp = psumT.tile([P, 2 * B], BF16, tag=f"tp{q}", name=f"tp{q}")
                nc.tensor.transpose(tp[:], h[:], id64[:])
                hT_self = hT_self_bufs[q][t % 2]
                nc.vector.tensor_copy(hT_self[:], tp[:])
                cc_in = dramp.tile([P, 2 * B], BF16, tag=f"ccin{q}",
                                   name=f"ccin{q}")
                nc.sync.dma_start(cc_in[:], hT_self[:])
                cc_out = dramp.tile([NCORES * P, 2 * B], BF16, tag=f"ccout{q}",
                                    name=f"ccout{q}")
                nc.gpsimd.collective_compute(
                    "AllGather",
                    mybir.AluOpType.bypass,
                    ins=[cc_in[:].opt()],
                    outs=[cc_out[:].opt()],
                    replica_groups=[list(range(NCORES))],
                )
                # reload split per k-chunk: 8 plain 2D DMAs land in parallel
                for k in range(KCH):
                    nc.sync.dma_start(
                        hT_recv_bufs[q][t % 2][:, k, :],
                        cc_out[k * P:(k + 1) * P, :],
                    )

            for t in range(S):
                for q in range(NQ):
                    step(q, t)
                if XPROJ_INTERLEAVE:
                    md = next(prod_iter, None)
                    if md is not None:
                        xproj_half(*md)

    if comm == "rdma":
        _strip_fake_incs(nc, ("recv_sem", "rel_sem"))
    return nc


def _strip_fake_incs(nc, sem_names):
    """Remove every on_update entry for the given sems (scheduler-sim-only
    stand-ins for remote increments) and drop update-only EventSemaphore
    carriers that become empty."""
    names = set(sem_names)
    for fn in nc.m.functions:
        for bb in fn.blocks:
            new = []
            changed = False
            for ins in bb.instructions:
                si = ins.sync_info
                if si is not None and any(
                    u.ant_name in names for u in si.on_update
                ):
                    kept = [u for u in si.on_update if u.ant_name not in names]
                    ins.sync_info = mybir.SyncInfo(
                        on_wait=list(si.on_wait), on_update=kept
                    )
                    changed = True
                    if (
                        isinstance(ins, mybir.InstEventSemaphore)
                        and not kept
                        and not si.on_wait
                    ):
                        continue  # drop the empty carrier
                new.append(ins)
            if changed:
                try:
                    bb.instructions = new
                except Exception:
                    bb.instructions.clear()
                    bb.instructions.extend(new)


def fix_drain_waits(nc):
    """This walrus build allows only 1 sync-wait per instruction (2 on
    EventSemaphore).  Move excess waits onto EventSemaphore insts placed
    immediately before the instruction on the same engine."""
    ctr = 0
    for fn in nc.m.functions:
        for bb in fn.blocks:
            insts = list(bb.instructions)
            new = []
            changed = False
            for ins in insts:
                si = ins.sync_info
                if (
                    not isinstance(ins, mybir.InstEventSemaphore)
                    and si is not None
                    and len(si.on_wait) > 1
                ):
                    waits = list(si.on_wait)
                    keep, extra = waits[:1], waits[1:]
                    for i in range(0, len(extra), 2):
                        w = mybir.InstEventSemaphore(
                            name=f"I-dwfix-{ctr}",
                            engine=ins.engine,
                            ins=[],
                            outs=[],
                            sync_info=mybir.SyncInfo(
                                on_wait=extra[i : i + 2], on_update=[]
                            ),
                        )
                        ctr += 1
                        new.append(w)
                    ins.sync_info = mybir.SyncInfo(
                        on_wait=keep, on_update=list(si.on_update)
                    )
                    changed = True
                new.append(ins)
            if changed:
                try:
                    bb.instructions = new
                except Exception:
                    bb.instructions.clear()
                    bb.instructions.extend(new)


def kernel(x, W_ii, W_hi, b_i, W_ii_reverse, W_hi_reverse, b_i_reverse):
    """Full inputs in, full (B, S, 2H) fp32 output out."""
    import os

    global LAST_EXEC_NS, LAST_RES
    import concourse.bass_utils as bu

    bu.upload_artifacts = lambda tmpdir: "local://" + tmpdir
    from concourse.bass_utils import run_bass_kernel_spmd

    S = S_FIXED
    comm = os.environ.get("TRNLSTM_COMM", KERNEL_COMM)
    trace = os.environ.get("TRNLSTM_TRACE", "0") == "1"
    global STREAMS
    STREAMS = int(os.environ.get("TRNLSTM_STREAMS", STREAMS))

    nc = build_kernel(S, comm=comm)
    nc.compile()
    fix_drain_waits(nc)
    in_maps = host_prep(x, W_ii, W_hi, b_i,
                        W_ii_reverse, W_hi_reverse, b_i_reverse, S)
    res = run_bass_kernel_spmd(nc, in_maps, list(range(NCORES)), trace=trace)
    LAST_EXEC_NS = res.exec_time_ns
    LAST_RES = res
    return host_assemble(res.results, S)


# revision 31
# speedup vs baseline: 1.0850x; 1.0594x over previous
"""BiLSTM (B=32, S=512, I=H=1024) Trainium2 kernel over 8 NeuronCores.

Strategy (v2): tensor-parallel over the gate dimension — each core owns a
128-row H-slice and its four gate blocks for BOTH directions.  The two
directions are computed together each step:

  - PSUM gates tile (64, 512) fp32: partitions 0-31 = fwd batch,
    32-63 = bwd batch; free dim = [i|f|o|g] x 128 (this core's units).
    fwd accumulates on PE column-group 0 (tile_position (*,0)), bwd on
    column-group 1 (tile_position (*,32)) -> the two 9-matmul chains run
    CONCURRENTLY on disjoint 32-column strips of the PE array.
  - All matmul operands are bf16 (weights, h, xp, identities); PSUM/state
    stay fp32.  Native Tanh for g and c (sigmoid+tanh share an ACT table
    set, so no reload).
  - Per step ONE exchange of hT (128 units, 64 = fwd|bwd batch) bf16:
    either a gpsimd remote-DMA broadcast to the 7 peers (comm="rdma") or a
    single ncfw AllGather (comm="ag").
  - x-projection for all steps is computed on-device first (phase B) into
    DRAM xp (bf16), bias folded in.
"""

KERNEL_COMM = "ag"  # "rdma" | "ag"  (rdma: SWDGE remote-DMA faults under axon)
S_FIXED = 512
XPROJ_INTERLEAVE = True  # compute x-projection chunks inside the step loop
XPROJ_LEAD = 10          # chunks precomputed per end before the recurrence
STREAMS = 1              # independent batch streams; >1 does NOT help: each
                         # stream still needs S sequential iterations, so total
                         # time = S * loop_latency regardless (measured)

LAST_EXEC_NS = None
LAST_RES = None

import numpy as np

import concourse.bass as bass
import concourse.bacc as bacc
import concourse.mybir as mybir
import concourse.tile as tile
from concourse import library_config
from concourse.tile_rust import add_dep_helper

# The axon client has no /dev/neuron*, so the driver's NC/routing maps are
# unavailable.  Relative-dest remote DMA descriptors don't bake these values
# into the NEFF, so a plausible identity map is fine for client-side
# validation and the simulator.
import concourse.libnrt as _libnrt

try:
    _libnrt.get_trn2_nc_mapping()
except Exception:
    _libnrt.get_trn2_nc_mapping = lambda: {(0, i): i for i in range(8)}
try:
    _libnrt.get_device_id_to_routing_id_mapping()
except Exception:
    _fake_rid_map = lambda: {i: i for i in range(16)}
    _libnrt.get_device_id_to_routing_id_mapping = _fake_rid_map
    import concourse.bass_interp as _bi
    import concourse.replica_groups as _rg

    _bi.get_device_id_to_routing_id_mapping = _fake_rid_map
    _rg.get_device_id_to_routing_id_mapping = _fake_rid_map

P = 128
B = 32
I_DIM = 1024
H_DIM = 1024
NCORES = 8
KCH = H_DIM // P          # 8 k-chunks of the hidden dim
GS = 4 * H_DIM // NCORES  # 512 gate rows per core per direction
F32 = mybir.dt.float32
BF16 = mybir.dt.bfloat16
SIG = mybir.ActivationFunctionType.Sigmoid
TANH = mybir.ActivationFunctionType.Tanh


def _bf16(a):
    import ml_dtypes

    return np.asarray(a, np.float32).astype(ml_dtypes.bfloat16)


def host_prep(x, W_ii, W_hi, b_i, W_ii_r, W_hi_r, b_i_r, S):
    """Build the 8 per-core input maps (everything bf16)."""
    x = np.asarray(x, np.float32)
    # xT[i, s*B+b] = x[b, s, i]
    xT = _bf16(np.ascontiguousarray(x.transpose(2, 1, 0).reshape(I_DIM, S * B)))

    def slices(W, bvec, core):
        # gate rows for this core, order [i|f|o|g] (no scaling: native tanh)
        rows_i = np.arange(core * P, core * P + P)
        rows = np.concatenate(
            [rows_i, H_DIM + rows_i, 3 * H_DIM + rows_i, 2 * H_DIM + rows_i]
        )
        Ws = np.asarray(W, np.float32)[rows, :]
        bs = np.asarray(bvec, np.float32)[rows]
        return _bf16(np.ascontiguousarray(Ws.T)), _bf16(bs.reshape(1, GS))

    # injection identity: strip s (rows 0.. / 32..) holds [I_SM | 0] so the
    # M=32-wide inject also zero-fills junk rows when STREAMS=2
    SM = B // STREAMS
    id2 = np.zeros((2 * B, B), np.float32)
    id2[0:SM, 0:SM] = np.eye(SM)
    id2[B:B + SM, 0:SM] = np.eye(SM)
    if STREAMS == 1:
        pass  # SM == B: strips fully identity, same as before
    id2 = _bf16(id2)
    id64 = _bf16(np.eye(2 * B))                           # (64, 64)
    ones128 = _bf16(np.ones((1, P)))
    in_maps = []
    for c in range(NCORES):
        wiT_f, bias_f = slices(W_ii, b_i, c)
        whT_f, _ = slices(W_hi, b_i, c)
        wiT_b, bias_b = slices(W_ii_r, b_i_r, c)
        whT_b, _ = slices(W_hi_r, b_i_r, c)
        in_maps.append({
            "xT": xT,
            "wiT_f": wiT_f, "whT_f": whT_f, "bias_f": bias_f,
            "wiT_b": wiT_b, "whT_b": whT_b, "bias_b": bias_b,
            "id2": id2, "id64": id64, "ones128": ones128,
        })
    return in_maps


def host_assemble(results, S):
    """results[c]["out"]: (2, S, B, P) bf16 -> full (B, S, 2H) fp32."""
    out = np.empty((B, S, 2 * H_DIM), np.float32)
    for c in range(NCORES):
        o = np.asarray(results[c]["out"], np.float32)  # (2, S, B, P)
        out[:, :, c * P:(c + 1) * P] = o[0].transpose(1, 0, 2)
        out[:, :, H_DIM + c * P:H_DIM + (c + 1) * P] = o[1].transpose(1, 0, 2)
    return out


def build_kernel(S, comm="rdma", rel_wait=False):
    nc = bacc.Bacc(None)
    SB = S * B
    MCH = SB // P  # sb-chunks of 128 (= 4 timesteps each)

    xT_e = nc.declare_dram_parameter("xT", [I_DIM, SB], BF16, isOutput=False)
    w_e = {}
    for d in ("f", "b"):
        w_e["wiT_" + d] = nc.declare_dram_parameter("wiT_" + d, [I_DIM, GS], BF16, isOutput=False)
        w_e["whT_" + d] = nc.declare_dram_parameter("whT_" + d, [H_DIM, GS], BF16, isOutput=False)
        w_e["bias_" + d] = nc.declare_dram_parameter("bias_" + d, [1, GS], BF16, isOutput=False)
    id2_e = nc.declare_dram_parameter("id2", [2 * B, B], BF16, isOutput=False)
    id64_e = nc.declare_dram_parameter("id64", [2 * B, 2 * B], BF16, isOutput=False)
    ones_e = nc.declare_dram_parameter("ones128", [1, P], BF16, isOutput=False)
    out_e = nc.declare_dram_parameter("out", [2, S, B, P], BF16, isOutput=True)

    MCH_ALL = S // 4
    # one DRAM tensor per 4-timestep chunk -> exact producer/consumer deps
    xp_chunks = [
        nc.dram_tensor(f"xp_{m}", [2, 4, B, GS], BF16) for m in range(MCH_ALL)
    ]

    with tile.TileContext(nc) as tc:
        with (
            tc.tile_pool(name="const", bufs=1) as constp,
            tc.tile_pool(name="xsb", bufs=3) as xsbp,
            tc.tile_pool(name="xpt_st", bufs=3) as xpst,
            tc.tile_pool(name="psumB", bufs=2, space="PSUM") as psumB,
            tc.tile_pool(name="psumC", bufs=2, space="PSUM") as psumC,
            tc.tile_pool(name="psumT", bufs=1, space="PSUM") as psumT,
            tc.tile_pool(name="state", bufs=1) as statep,
            tc.tile_pool(name="step", bufs=3) as stepp,
            tc.tile_pool(name="dram", bufs=2, space="DRAM") as dramp,
        ):
            if comm == "rdma":
                nc.gpsimd.load_library(library_config.remote_dma)
            # ---- constants / weights in SBUF ----
            idU = constp.tile([2 * B, B], BF16, tag="id2", name="id2")
            nc.sync.dma_start(idU[:], id2_e[:])
            id64 = constp.tile([2 * B, 2 * B], BF16, tag="id64", name="id64")
            nc.sync.dma_start(id64[:], id64_e[:])
            ones128 = constp.tile([1, P], BF16, tag="ones", name="ones")
            nc.sync.dma_start(ones128[:], ones_e[:])
            wiT = {}
            whT = {}
            biasT = {}
            for d in ("f", "b"):
                wiT[d] = constp.tile([P, KCH, GS], BF16, tag="wiT" + d, name="wiT" + d)
                nc.sync.dma_start(
                    wiT[d][:],
                    w_e["wiT_" + d][:].rearrange("(k p) g -> p k g", p=P),
                )
                whT[d] = constp.tile([P, KCH, GS], BF16, tag="whT" + d, name="whT" + d)
                nc.sync.dma_start(
                    whT[d][:],
                    w_e["whT_" + d][:].rearrange("(k p) g -> p k g", p=P),
                )
                biasT[d] = constp.tile([1, GS], BF16, tag="bias" + d, name="bias" + d)
                nc.sync.dma_start(biasT[d][:], w_e["bias_" + d][:])

            # ---- x_proj of one 4-timestep chunk into DRAM (bias folded) ----
            _xsb_cur = {}

            def xproj_half(m, d):
                if d == "f":
                    xsb = xsbp.tile([P, KCH, P], BF16, tag="xsb", name="xsb")
                    nc.sync.dma_start(
                        xsb[:],
                        xT_e[:, m * P:(m + 1) * P].rearrange(
                            "(k p) c -> p k c", p=P),
                    )
                    _xsb_cur[m] = xsb
                else:
                    xsb = _xsb_cur.pop(m)
                ps = psumB.tile([P, GS], F32, tag="psB", name="psB")
                nc.tensor.matmul(ps[:], ones128[:], biasT[d][:],
                                 start=True, stop=False)
                for k in range(KCH):
                    nc.tensor.matmul(ps[:], xsb[:, k, :], wiT[d][:, k, :],
                                     start=False, stop=(k == KCH - 1))
                xpt = xpst.tile([P, GS], BF16, tag="xpt", name="xpt")
                nc.vector.tensor_copy(xpt[:], ps[:])
                di = 0 if d == "f" else 1
                nc.sync.dma_start(
                    xp_chunks[m][di].rearrange("s b g -> (s b) g"),
                    xpt[:],
                )

            def xproj_chunk(m):
                xproj_half(m, "f")
                xproj_half(m, "b")

            # chunk production order: LEAD chunks from each end up front, the
            # rest interleaved into the recurrence (front/back alternating)
            lead = min(XPROJ_LEAD, (MCH_ALL + 1) // 2) if XPROJ_INTERLEAVE else 0
            if not XPROJ_INTERLEAVE:
                for m in range(MCH_ALL):
                    xproj_chunk(m)
            else:
                for i in range(lead):
                    xproj_chunk(i)
                    if MCH_ALL - 1 - i > lead - 1:
                        xproj_chunk(MCH_ALL - 1 - i)
                pending = list(range(lead, MCH_ALL - lead))
                # alternate front/back consumption order
                prod_order = []
                lo, hi = 0, len(pending) - 1
                while lo <= hi:
                    prod_order.append(pending[lo]); lo += 1
                    if lo <= hi:
                        prod_order.append(pending[hi]); hi -= 1
                prod_iter = iter(
                    [(m, d) for m in prod_order for d in ("f", "b")])

            # ---- phase C: recurrence, both dirs together, NQ streams ----
            NQ = STREAMS
            SM = B // NQ  # batch rows per stream per direction
            c_state = {}
            hT_self_bufs = {}
            hT_recv_bufs = {}
            for q in range(NQ):
                c_state[q] = statep.tile([2 * B, P], F32, tag=f"c{q}",
                                         name=f"c{q}")
                nc.vector.memset(c_state[q][:], 0.0)
                hT_self_bufs[q] = [
                    statep.tile([P, 2 * B], BF16, tag=f"hTs{q}{p}",
                                name=f"hTs{q}{p}") for p in (0, 1)
                ]
                hT_recv_bufs[q] = [
                    statep.tile([P, KCH, 2 * B], BF16, tag=f"hTr{q}{p}",
                                name=f"hTr{q}{p}") for p in (0, 1)
                ]

            def step(q, t):
                # xp for fwd step t and bwd step t (source position S-1-t),
                # this stream's batch slice; junk lanes are zero-filled by the
                # M=32-wide injection matmuls so downstream math stays finite.
                sb = S - 1 - t
                bs = slice(SM * q, SM * (q + 1))
                xpt = stepp.tile([2 * B, GS], BF16, tag=f"xp_t{q}",
                                 name=f"xp_t{q}")
                # NOTE: must stay on the same (sync) queue as the xp chunk
                # writes — DRAM RAW deps are not tracked, FIFO order is the
                # only thing keeping producer before consumer.
                nc.sync.dma_start(xpt[0:SM, :], xp_chunks[t // 4][0, t % 4, bs])
                nc.sync.dma_start(xpt[B:B + SM, :],
                                  xp_chunks[sb // 4][1, sb % 4, bs])
                ps = psumC.tile([2 * B, GS], F32, tag=f"psC{q}", name=f"psC{q}")
                # xp injection via identity matmuls (row strips 0 / 1);
                # idU is [I_SM | 0] per strip -> writes zeros to junk rows
                nc.tensor.matmul(ps[0:B, :], idU[0:SM, :], xpt[0:SM, :],
                                 start=True, stop=(t == 0),
                                 tile_position=(0, 0))
                nc.tensor.matmul(ps[B:2 * B, :], idU[B:B + SM, :],
                                 xpt[B:B + SM, :],
                                 start=True, stop=(t == 0),
                                 tile_position=(B, B))
                if t > 0:
                    hT = hT_recv_bufs[q][(t - 1) % 2]
                    for k in range(KCH):
                        nc.tensor.matmul(
                            ps[0:SM, :], hT[:, k, 0:SM], whT["f"][:, k, :],
                            start=False, stop=(k == KCH - 1),
                            tile_position=(0, 0))
                        nc.tensor.matmul(
                            ps[B:B + SM, :], hT[:, k, B:B + SM],
                            whT["b"][:, k, :],
                            start=False, stop=(k == KCH - 1),
                            tile_position=(0, B))
                # activations: g(tanh) and i(sig) first so the DVE chain can
                # start while [f|o] sigmoids run
                acts = stepp.tile([2 * B, GS], F32, tag=f"acts{q}",
                                  name=f"acts{q}")
                nc.scalar.activation(acts[:, 3 * P:4 * P], ps[:, 3 * P:4 * P],
                                     TANH)
                nc.scalar.activation(acts[:, 0:P], ps[:, 0:P], SIG)
                nc.scalar.activation(acts[:, P:3 * P], ps[:, P:3 * P], SIG)
                i_ap = acts[:, 0 * P:1 * P]
                f_ap = acts[:, 1 * P:2 * P]
                o_ap = acts[:, 2 * P:3 * P]
                g_ap = acts[:, 3 * P:4 * P]
                u = stepp.tile([2 * B, P], F32, tag=f"u{q}", name=f"u{q}")
                nc.vector.tensor_mul(u[:], i_ap, g_ap)
                v = stepp.tile([2 * B, P], F32, tag=f"v{q}", name=f"v{q}")
                nc.vector.tensor_mul(v[:], f_ap, c_state[q][:])
                nc.vector.tensor_add(c_state[q][:], u[:], v[:])
                tc_t = stepp.tile([2 * B, P], F32, tag=f"tc{q}", name=f"tc{q}")
                nc.scalar.activation(tc_t[:], c_state[q][:], TANH)
                h = stepp.tile([2 * B, P], BF16, tag=f"h{q}", name=f"h{q}")
                nc.vector.tensor_mul(h[:], o_ap, tc_t[:])
                nc.scalar.dma_start(out_e[0, t, bs], h[0:SM, :])
                nc.scalar.dma_start(out_e[1, S - 1 - t, bs], h[B:B + SM, :])
                if t == S - 1:
                    return
                # transpose h -> (P, 2B) and publish to peers
                tp = psumT.tile([P, 2 * B], BF16, tag=f"tp{q}", name=f"tp{q}")
                nc.tensor.transpose(tp[:], h[:], id64[:])
                hT_self = hT_self_bufs[q][t % 2]
                nc.vector.tensor_copy(hT_self[:], tp[:])
                cc_in = dramp.tile([P, 2 * B], BF16, tag=f"ccin{q}",
                                   name=f"ccin{q}")
                nc.sync.dma_start(cc_in[:], hT_self[:])
                cc_out = dramp.tile([NCORES * P, 2 * B], BF16, tag=f"ccout{q}",
                                    name=f"ccout{q}")
                nc.gpsimd.collective_compute(
                    "AllGather",
                    mybir.AluOpType.bypass,
                    ins=[cc_in[:].opt()],
                    outs=[cc_out[:].opt()],
                    replica_groups=[list(range(NCORES))],
                )
                # reload in two halves: first 4 chunks land while later MMs
                # still queue, without paying 8x Sync-queue issue cost
                for g in (0, 1):
                    nc.sync.dma_start(
                        hT_recv_bufs[q][t % 2][:, 4 * g:4 * g + 4, :],
                        cc_out[4 * g * P:4 * (g + 1) * P, :].rearrange(
                            "(k p) b -> p k b", p=P),
                    )

            warm_sink = statep.tile([B, 1], F32, tag="wsink", name="wsink")

            def pe_warm(n):
                """Chain of n dummy matmuls that drain during the collective
                window, keeping the PE's HAM clock at 8/8 so the
                latency-critical recurrence matmuls run at 2.4 GHz.  The tiny
                DVE copy consumes the psum so the chain isn't eliminated."""
                wps = psumB.tile([B, GS], F32, tag="warm", name="warm")
                for j in range(n):
                    nc.tensor.matmul(wps[:], whT["f"][:, 0, 0:B],
                                     wiT["f"][:, 0, :],
                                     start=(j == 0), stop=(j == n - 1))
                nc.vector.tensor_copy(warm_sink[:], wps[:, 0:1])

            for t in range(S):
                for q in range(NQ):
                    step(q, t)
                did_xproj = False
                if XPROJ_INTERLEAVE:
                    md = next(prod_iter, None)
                    if md is not None:
                        xproj_half(*md)
                        did_xproj = True
                if t < S - 1:
                    pe_warm(8 if did_xproj else 16)

    if comm == "rdma":
        _strip_fake_incs(nc, ("recv_sem", "rel_sem"))
    return nc


def _strip_fake_incs(nc, sem_names):
    """Remove every on_update entry for the given sems (scheduler-sim-only
    stand-ins for remote increments) and drop update-only EventSemaphore
    carriers that become empty."""
    names = set(sem_names)
    for fn in nc.m.functions:
        for bb in fn.blocks:
            new = []
            changed = False
            for ins in bb.instructions:
                si = ins.sync_info
                if si is not None and any(
                    u.ant_name in names for u in si.on_update
                ):
                    kept = [u for u in si.on_update if u.ant_name not in names]
                    ins.sync_info = mybir.SyncInfo(
                        on_wait=list(si.on_wait), on_update=kept
                    )
                    changed = True
                    if (
                        isinstance(ins, mybir.InstEventSemaphore)
                        and not kept
                        and not si.on_wait
                    ):
                        continue  # drop the empty carrier
                new.append(ins)
            if changed:
                try:
                    bb.instructions = new
                except Exception:
                    bb.instructions.clear()
                    bb.instructions.extend(new)


def fix_drain_waits(nc):
    """This walrus build allows only 1 sync-wait per instruction (2 on
    EventSemaphore).  Move excess waits onto EventSemaphore insts placed
    immediately before the instruction on the same engine."""
    ctr = 0
    for fn in nc.m.functions:
        for bb in fn.blocks:
            insts = list(bb.instructions)
            new = []
            changed = False
            for ins in insts:
                si = ins.sync_info
                if (
                    not isinstance(ins, mybir.InstEventSemaphore)
                    and si is not None
                    and len(si.on_wait) > 1
                ):
                    waits = list(si.on_wait)
                    keep, extra = waits[:1], waits[1:]
                    for i in range(0, len(extra), 2):
                        w = mybir.InstEventSemaphore(
                            name=f"I-dwfix-{ctr}",
                            engine=ins.engine,
                            ins=[],
                            outs=[],
                            sync_info=mybir.SyncInfo(
                                on_wait=extra[i : i + 2], on_update=[]
                            ),
                        )
                        ctr += 1
                        new.append(w)
                    ins.sync_info = mybir.SyncInfo(
                        on_wait=keep, on_update=list(si.on_update)
                    )
                    changed = True
                new.append(ins)
            if changed:
                try:
                    bb.instructions = new
                except Exception:
                    bb.instructions.clear()
                    bb.instructions.extend(new)


def kernel(x, W_ii, W_hi, b_i, W_ii_reverse, W_hi_reverse, b_i_reverse):
    """Full inputs in, full (B, S, 2H) fp32 output out."""
    import os

    global LAST_EXEC_NS, LAST_RES
    import concourse.bass_utils as bu

    bu.upload_artifacts = lambda tmpdir: "local://" + tmpdir
    from concourse.bass_utils import run_bass_kernel_spmd

    S = S_FIXED
    comm = os.environ.get("TRNLSTM_COMM", KERNEL_COMM)
    trace = os.environ.get("TRNLSTM_TRACE", "0") == "1"
    global STREAMS
    STREAMS = int(os.environ.get("TRNLSTM_STREAMS", STREAMS))

    nc = build_kernel(S, comm=comm)
    nc.compile()
    fix_drain_waits(nc)
    in_maps = host_prep(x, W_ii, W_hi, b_i,
                        W_ii_reverse, W_hi_reverse, b_i_reverse, S)
    res = run_bass_kernel_spmd(nc, in_maps, list(range(NCORES)), trace=trace)
    LAST_EXEC_NS = res.exec_time_ns
    LAST_RES = res
    return host_assemble(res.results, S)
